# revision 1
# baseline (speedup 1.0000x reference)
"""Trainium2 Bass kernel for nn_MultiHeadAttention_56066503082144.

Reference computation (per batch b):
  Q = relu(x @ Wq + bq), K = relu(x @ Wk + bk), V = relu(x @ Wv + bv)
  scores[b,h,q,k] = (Q_h @ K_h^T) / sqrt(dh)
  attn = softmax(scores, axis=q)            # NON-STANDARD: over the query axis
  out[b,q,:] = concat_h(attn_h @ V_h)
  y = out + x                               # residual
  y = batchnorm(y)                          # per-channel stats over (B, S)

Sharding: data-parallel over batch B=8 across the 8 NeuronCores (one batch
element per core).  Cross-core communication = one tiny AllReduce of the
BatchNorm partial sums (plus a warm-up AllReduce that eats launch skew).

Per-core structure (S=1024, D=512, H=8, dh=64):
  - x DMA'd naturally (split across the SP+ACT DMA queues), cast to bf16 on
    DVE, transposed on PE (bf16, 1 cyc/row) -> xT.
  - Q^T,K^T computed transposed (lhsT=W chunk, rhs=xT); bias+relu
    evacuation on ScalarE for m=0 (startup) and on DVE (fused
    tensor_scalar add+max) for m=1..3, which are projected lazily inside
    the attention loop so the first exp starts as early as possible.
  - V natural [s, e]; relu+bias evac on DVE.  V must NOT interleave with
    pair-0 score matmuls (observed PSUM corruption on even partitions when
    other matmul groups interleave the attention stream), so it sits
    between the QK[0] projection and the first score matmul.
  - Attention runs in HEAD PAIRS: the two heads of QT[m]/KT[m] live on
    partitions 0:64/64:128, so their dh=64-contraction score matmuls map to
    disjoint PE row groups, and the two attnV matmuls write disjoint col
    groups (auto tile_position from base partitions).
  - softmax over q = row-sum in the transposed score layout: exp on ScalarE
    (the bottleneck: 64 x [128,1024] activations) with the free accum_out
    row-sum; denominator folded into V' rows (DVE reciprocal + scale).
  - Residual + BN partial stats per pair on DVE (pair 3: square+accum on
    the now-idle ScalarE); ONE AllReduce of [128,8] partials at the end.
    A dummy warm-up AllReduce at kernel start absorbs inter-core launch
    skew and the CC dispatch latency; dummy Exp/Sqrt activations prefetch
    ACT table sets off the critical path.
  - BN affine applied in TRANSPOSED layout (per-partition scale/bias) on
    ScalarE with bf16 output, then transposed back on PE (bf16 stays fast
    on a cold PE clock), DVE-copied to fp32, DMA'd out.
"""

import math

import numpy as np

P = 128
D = 512
H = 8
DH = 64
S_FULL = 1024
B_FULL = 8
N_CORES = 8
BN_EPS = 1e-5

_CACHE = {}


class _Done(Exception):
    """Early-exit marker for phase-bisection builds."""


def _build(S=S_FULL, n_cores=N_CORES, total_tokens=None, stop_after="full"):
    import concourse.bacc as bacc
    import concourse.bass as bass
    import concourse.tile as tile
    from concourse import mybir
    from concourse.masks import make_identity

    f32 = mybir.dt.float32
    bf16 = mybir.dt.bfloat16
    AF = mybir.ActivationFunctionType
    ALU = mybir.AluOpType
    AX = mybir.AxisListType

    if total_tokens is None:
        total_tokens = n_cores * S
    inv_ntok = 1.0 / float(total_tokens)

    ND = D // P          # 4 d-chunks
    NS = S // P          # 8 s-chunks
    NPAIR = H // 2       # 4 head pairs == 4 d-chunks of the output
    inv_sqrt_dh = 1.0 / math.sqrt(DH)

    nc = bacc.Bacc(
        "TRN2",
        target_bir_lowering=False,
        debug=False,
        num_devices=n_cores,
    )

    x_d = nc.dram_tensor("x", [S, D], f32, kind="ExternalInput").ap()
    Wq_d = nc.dram_tensor("Wq", [D, D], f32, kind="ExternalInput").ap()
    bq_d = nc.dram_tensor("bq", [D], f32, kind="ExternalInput").ap()
    Wk_d = nc.dram_tensor("Wk", [D, D], f32, kind="ExternalInput").ap()
    bk_d = nc.dram_tensor("bk", [D], f32, kind="ExternalInput").ap()
    Wv_d = nc.dram_tensor("Wv", [D, D], f32, kind="ExternalInput").ap()
    bv_d = nc.dram_tensor("bv", [D], f32, kind="ExternalInput").ap()
    gamma_d = nc.dram_tensor("gamma", [D], f32, kind="ExternalInput").ap()
    beta_d = nc.dram_tensor("beta", [D], f32, kind="ExternalInput").ap()
    y_d = nc.dram_tensor("y", [S, D], f32, kind="ExternalOutput").ap()

    from contextlib import ExitStack

    with tile.TileContext(nc) as tc, ExitStack() as stk:
        consts = stk.enter_context(tc.tile_pool(name="consts", bufs=1))
        persist = stk.enter_context(tc.tile_pool(name="persist", bufs=1))
        work = stk.enter_context(tc.tile_pool(name="work", bufs=4))
        wstage = stk.enter_context(tc.tile_pool(name="wstage", bufs=6))
        xstage = stk.enter_context(tc.tile_pool(name="xstage", bufs=8))
        epool = stk.enter_context(tc.tile_pool(name="epool", bufs=6))
        outp = stk.enter_context(tc.tile_pool(name="outp", bufs=3))
        sqp = stk.enter_context(tc.tile_pool(name="sqp", bufs=2))
        # PSUM: pool_s 3x[128,1024]f32 = 6 banks, pool_o 1x[128,1024]f32 = 2
        pool_s = stk.enter_context(tc.tile_pool(name="psum_s", bufs=3, space="PSUM"))
        pool_o = stk.enter_context(tc.tile_pool(name="psum_o", bufs=1, space="PSUM"))
        dram = stk.enter_context(tc.tile_pool(name="dram", bufs=1, space="DRAM"))

        # ---------- constants ----------
        ident_f = consts.tile([P, P], f32)
        make_identity(nc, ident_f)
        ident_b = consts.tile([P, P], bf16)
        nc.gpsimd.tensor_copy(ident_b, ident_f)

        # per-partition (transposed-layout) vectors [128, ND]
        bqT = consts.tile([P, ND], f32)
        nc.gpsimd.dma_start(out=bqT, in_=bq_d.rearrange("(m p) -> p m", p=P))
        bkT = consts.tile([P, ND], f32)
        nc.gpsimd.dma_start(out=bkT, in_=bk_d.rearrange("(m p) -> p m", p=P))
        gT = consts.tile([P, ND], f32)
        nc.gpsimd.dma_start(out=gT, in_=gamma_d.rearrange("(m p) -> p m", p=P))
        betaT = consts.tile([P, ND], f32)
        nc.gpsimd.dma_start(out=betaT, in_=beta_d.rearrange("(m p) -> p m", p=P))
        # bv broadcast across partitions [128, D] for the V bias add
        bvb = consts.tile([P, D], f32)
        bv_bc = bass.AP(tensor=bv_d.tensor, offset=bv_d.offset,
                        ap=[[0, P]] + list(bv_d.ap))
        nc.gpsimd.dma_start(out=bvb, in_=bv_bc)
        # scratch for dummy activations that prefetch ACT table sets
        actpin = consts.tile([1, 1], f32)
        nc.vector.memset(actpin, 1.0)
        # Warm-up AllReduce: absorbs inter-core launch skew and pays the
        # CC dispatch latency early, so the real stats AllReduce at the end
        # only waits for residual drift.  Result is never read.
        warm_in = dram.tile([1, 1], f32)
        warm_out = dram.tile(
            [1, 1], f32, addr_space="Shared" if n_cores > 4 else "Local")
        nc.gpsimd.dma_start(out=warm_in, in_=actpin)
        nc.gpsimd.collective_compute(
            "AllReduce",
            ALU.add,
            replica_groups=[list(range(n_cores))],
            ins=[warm_in.opt()],
            outs=[warm_out.opt()],
        )

        # ---------- load x (sync queue) and weights (scalar queue) ----------
        xf = []
        for i in range(NS):
            t = xstage.tile([P, D], f32, tag="xf")
            xeng = nc.sync if i % 2 == 0 else nc.scalar
            xeng.dma_start(out=t, in_=x_d[i * P:(i + 1) * P, :])
            xf.append(t)
        wsb = {}
        wf_stage = {}
        for nm, wd in (("q", Wq_d), ("k", Wk_d), ("v", Wv_d)):
            fs, bs = [], []
            for k in range(ND):
                tf = wstage.tile([P, D], f32, tag="wf")
                weng = nc.gpsimd if nm == "v" else nc.scalar
                weng.dma_start(out=tf, in_=wd[k * P:(k + 1) * P, :])
                fs.append(tf)
                bs.append(persist.tile([P, D], bf16, name=f"W{nm}{k}",
                                       tag=f"W{nm}{k}"))
            wf_stage[nm] = fs
            wsb[nm] = bs

        # ---------- casts on DVE (fast) ----------
        xb = [persist.tile([P, D], bf16, name=f"xb{i}", tag=f"xb{i}")
              for i in range(NS)]
        for i in range(NS):
            nc.vector.tensor_copy(xb[i], xf[i])
        for nm in ("q", "k", "v"):
            for k in range(ND):
                nc.vector.tensor_copy(wsb[nm][k], wf_stage[nm][k])

        # ---------- transpose x -> xT bf16 (PE, 1 cyc/row) ----------
        xT = [persist.tile([P, S], bf16, name=f"xT{j}", tag=f"xT{j}")
              for j in range(ND)]
        for j in range(ND):
            pt = pool_s.tile([P, S], bf16, tag="ps")
            for i in range(NS):
                nc.tensor.transpose(
                    pt[:, i * P:(i + 1) * P],
                    xb[i][:, j * P:(j + 1) * P],
                    ident_b,
                )
            nc.vector.tensor_copy(xT[j], pt)

        # ---------- projections ----------
        QT = [persist.tile([P, S], bf16, name=f"QT{m}", tag=f"QT{m}")
              for m in range(ND)]
        KT = [persist.tile([P, S], bf16, name=f"KT{m}", tag=f"KT{m}")
              for m in range(ND)]

        def emit_qk_proj(m, on_scalar):
            """QT[m], KT[m] = relu(W^T x^T + b); evac on ScalarE or DVE."""
            for dst, wname, bT in ((QT, "q", bqT), (KT, "k", bkT)):
                pq = pool_s.tile([P, S], f32, tag="ps")
                for n in range(2):
                    for k in range(ND):
                        nc.tensor.matmul(
                            pq[:, n * 512:(n + 1) * 512],
                            lhsT=wsb[wname][k][:, m * P:(m + 1) * P],
                            rhs=xT[k][:, n * 512:(n + 1) * 512],
                            start=(k == 0), stop=(k == ND - 1),
                        )
                if on_scalar:
                    nc.scalar.activation(
                        out=dst[m], in_=pq, func=AF.Relu, bias=bT[:, m:m + 1])
                else:
                    nc.vector.tensor_scalar(
                        out=dst[m], in0=pq, scalar1=bT[:, m:m + 1],
                        scalar2=0.0, op0=ALU.add, op1=ALU.max)

        emit_qk_proj(0, on_scalar=True)
        # prefetch the exp table set during startup (off the critical path)
        nc.scalar.activation(out=actpin, in_=actpin, func=AF.Exp)

        # V natural (bias via 1-partition matmul, relu evac on DVE)
        V_nat = [persist.tile([P, D], bf16, name=f"V{i}", tag=f"V{i}")
                 for i in range(NS)]

        def emit_v(i):
            pv = pool_s.tile([P, S], f32, tag="ps")
            for k in range(ND):
                nc.tensor.matmul(
                    pv[:, :D],
                    lhsT=xT[k][:, i * P:(i + 1) * P],
                    rhs=wsb["v"][k],
                    start=(k == 0), stop=(k == ND - 1),
                )
            vb_t = work.tile([P, D], f32, tag="vbt")
            nc.vector.tensor_add(vb_t, pv[:, :D], bvb)
            nc.vector.tensor_scalar_max(V_nat[i], vb_t, 0.0)

        # ---------- attention, head-pair by head-pair ----------
        yT = [persist.tile([P, S], f32, name=f"yT{m}", tag=f"yT{m}")
              for m in range(ND)]
        # BN partials: [sum, sumsq] per pair packed into one [P, 8] tile
        stp = consts.tile([P, 2 * NPAIR], f32)
        stg = consts.tile([P, 2 * NPAIR], f32)
        stats_in_a = dram.tile([P, 2 * NPAIR], f32)
        stats_out_a = dram.tile(
            [P, 2 * NPAIR], f32,
            addr_space="Shared" if n_cores > 4 else "Local")

        for pair in range(NPAIR):
            QA = QT[pair][0:DH, :]
            QB = QT[pair][DH:P, :]
            KA = KT[pair][0:DH, :]
            KB = KT[pair][DH:P, :]
            po = pool_o.tile([P, S], f32, tag="po")
            eA = [None] * NS
            eB = [None] * NS
            vpA = [None] * NS
            vpB = [None] * NS

            def emit_scores(kc, pair=pair, QA=QA, QB=QB, KA=KA, KB=KB,
                            eA=eA, eB=eB, vpA=vpA, vpB=vpB):
                sA = pool_s.tile([P, S], f32, tag="ps")
                sB = pool_s.tile([P, S], f32, tag="ps")
                for n in range(2):
                    nc.tensor.matmul(sA[:, n * 512:(n + 1) * 512],
                                     lhsT=KA[:, kc * P:(kc + 1) * P],
                                     rhs=QA[:, n * 512:(n + 1) * 512],
                                     start=True, stop=True)
                    nc.tensor.matmul(sB[:, n * 512:(n + 1) * 512],
                                     lhsT=KB[:, kc * P:(kc + 1) * P],
                                     rhs=QB[:, n * 512:(n + 1) * 512],
                                     start=True, stop=True)
                ea = epool.tile([P, S], bf16, tag="E")
                rsA = work.tile([P, 1], f32, tag="rsA")
                nc.scalar.activation(out=ea, in_=sA, func=AF.Exp,
                                     scale=inv_sqrt_dh, accum_out=rsA)
                eb = epool.tile([P, S], bf16, tag="E")
                rsB = work.tile([P, 1], f32, tag="rsB")
                nc.scalar.activation(out=eb, in_=sB, func=AF.Exp,
                                     scale=inv_sqrt_dh, accum_out=rsB)
                eA[kc] = ea
                eB[kc] = eb
                rrA = work.tile([P, 1], f32, tag="rrA")
                nc.vector.reciprocal(rrA, rsA)
                rrB = work.tile([P, 1], f32, tag="rrB")
                nc.vector.reciprocal(rrB, rsB)
                hA = 2 * pair
                hB = 2 * pair + 1
                va = work.tile([P, DH], bf16, tag="vpA")
                nc.vector.tensor_scalar_mul(
                    va, V_nat[kc][:, hA * DH:(hA + 1) * DH], rrA)
                vb = work.tile([P, DH], bf16, tag="vpB")
                nc.vector.tensor_scalar_mul(
                    vb, V_nat[kc][:, hB * DH:(hB + 1) * DH], rrB)
                vpA[kc] = va
                vpB[kc] = vb

            def emit_av(kc, po=po, eA=eA, eB=eB, vpA=vpA, vpB=vpB):
                for n in range(2):
                    nc.tensor.matmul(po[0:DH, n * 512:(n + 1) * 512],
                                     lhsT=vpA[kc],
                                     rhs=eA[kc][:, n * 512:(n + 1) * 512],
                                     start=(kc == 0), stop=(kc == NS - 1))
                    nc.tensor.matmul(po[DH:P, n * 512:(n + 1) * 512],
                                     lhsT=vpB[kc],
                                     rhs=eB[kc][:, n * 512:(n + 1) * 512],
                                     start=(kc == 0), stop=(kc == NS - 1))

            if pair == 0:
                for i in range(NS):
                    emit_v(i)
            for kc in range(NS):
                emit_scores(kc)
                if pair < NPAIR - 1 and kc == 1:
                    emit_qk_proj(pair + 1, on_scalar=False)
                if kc >= 2:
                    emit_av(kc - 2)
            for kc in range(max(NS - 2, 0), NS):
                emit_av(kc)

            # residual add (PSUM + xT -> yT), BN partials, per-pair AllReduce
            nc.vector.tensor_add(yT[pair], po, xT[pair])
            nc.vector.tensor_reduce(
                out=stp[:, 2 * pair:2 * pair + 1], in_=yT[pair],
                axis=AX.X, op=ALU.add)
            if pair == NPAIR - 1:
                # ScalarE is idle after the exp marathon: square+accumulate
                # there to shorten the serial DVE chain before the AllReduce
                sqs = sqp.tile([P, S], f32, tag="sq")
                nc.scalar.activation(
                    out=sqs, in_=yT[pair], func=AF.Square,
                    accum_out=stp[:, 2 * pair + 1:2 * pair + 2])
            else:
                sq = sqp.tile([P, S], f32, tag="sq")
                nc.vector.tensor_mul(sq, yT[pair], yT[pair])
                nc.vector.tensor_reduce(
                    out=stp[:, 2 * pair + 1:2 * pair + 2], in_=sq,
                    axis=AX.X, op=ALU.add)
            nc.gpsimd.dma_start(
                out=stats_in_a[:, 2 * pair:2 * pair + 2],
                in_=stp[:, 2 * pair:2 * pair + 2])
            if pair == NPAIR - 1:
                # prefetch the sqrt table set while the AllReduce drains
                nc.scalar.activation(out=actpin, in_=actpin, func=AF.Sqrt)
                nc.gpsimd.collective_compute(
                    "AllReduce",
                    ALU.add,
                    replica_groups=[list(range(n_cores))],
                    ins=[stats_in_a.opt()],
                    outs=[stats_out_a.opt()],
                )
                nc.gpsimd.dma_start(out=stg, in_=stats_out_a)

        if stop_after in ("yt", "qk", "kt"):
            # transposed-layout dumps: y_d[s, d] viewed as d-major strips
            for m in range(ND):
                y_strip = bass.AP(tensor=y_d.tensor, offset=m * P,
                                  ap=[[1, P], [D, S]])
                if stop_after == "yt":
                    nc.sync.dma_start(out=y_strip, in_=yT[m])
                else:
                    src = QT[m] if stop_after == "qk" else KT[m]
                    zf = outp.tile([P, S], f32, tag="zf")
                    nc.vector.tensor_copy(zf, src)
                    nc.sync.dma_start(out=y_strip, in_=zf)
        if stop_after == "v":
            for i in range(NS):
                vf = outp.tile([P, D], f32, tag="vf")
                nc.vector.tensor_copy(vf, V_nat[i])
                nc.sync.dma_start(out=y_d[i * P:(i + 1) * P, :], in_=vf)

        if stop_after == "full":
            # ---------- BN coefficients ----------
            # A = gamma * rsqrt(var+eps), C = beta - mean*A   (transposed layout)
            ac = consts.tile([P, 2 * ND], f32)   # cols 0..3 = A, 4..7 = C
            mean = consts.tile([P, ND], f32)
            var = consts.tile([P, ND], f32)
            stg3 = stg.rearrange("p (m two) -> p m two", two=2)
            nc.vector.tensor_scalar_mul(mean, stg3[:, :, 0], inv_ntok)
            nc.vector.tensor_scalar_mul(var, stg3[:, :, 1], inv_ntok)
            m2 = consts.tile([P, ND], f32)
            nc.vector.tensor_mul(m2, mean, mean)
            nc.vector.tensor_sub(var, var, m2)
            epsT = consts.tile([P, 1], f32)
            nc.vector.memset(epsT, BN_EPS)
            sd = consts.tile([P, ND], f32)
            nc.scalar.activation(out=sd, in_=var, func=AF.Sqrt, bias=epsT)
            rsd = consts.tile([P, ND], f32)
            nc.vector.reciprocal(rsd, sd)
            nc.vector.tensor_mul(ac[:, 0:ND], gT, rsd)
            nc.vector.tensor_mul(ac[:, ND:2 * ND], mean, ac[:, 0:ND])
            nc.vector.tensor_sub(ac[:, ND:2 * ND], betaT, ac[:, ND:2 * ND])

            # ---------- affine in TRANSPOSED layout (per-partition A,C) ----
            # z[m] = A[m]*yT[m] + C[m] on the (idle) ScalarE, bf16 out so the
            # transposes run at 1 cyc/row even on a cold PE clock.
            zT = [persist.tile([P, S], bf16, name=f"zT{m}", tag=f"zT{m}")
                  for m in range(ND)]
            for m in range(ND):
                nc.scalar.activation(
                    out=zT[m], in_=yT[m], func=AF.Identity,
                    scale=ac[:, m:m + 1], bias=ac[:, ND + m:ND + m + 1])

            # ---------- transpose back + DMA out ----------
            for i in range(NS):
                pz = pool_s.tile([P, S], bf16, tag="ps")
                for m in range(ND):
                    nc.tensor.transpose(
                        pz[:, m * P:(m + 1) * P],
                        zT[m][:, i * P:(i + 1) * P],
                        ident_b,
                    )
                yo = outp.tile([P, D], f32, tag="yo")
                nc.vector.tensor_copy(yo, pz[:, :D])
                nc.sync.dma_start(out=y_d[i * P:(i + 1) * P, :], in_=yo)

    nc.compile()
    return nc


def _get_program(S=S_FULL, n_cores=N_CORES, total_tokens=None):
    key = (S, n_cores, total_tokens)
    if key not in _CACHE:
        _CACHE[key] = _build(S, n_cores, total_tokens)
    return _CACHE[key]


def kernel(**inputs):
    x = np.ascontiguousarray(np.asarray(inputs["x"], dtype=np.float32))
    B, S, Dx = x.shape
    assert (B, S, Dx) == (B_FULL, S_FULL, D), (B, S, Dx)
    names = ["Wq", "bq", "Wk", "bk", "Wv", "bv", "gamma", "beta"]
    shared = {
        n: np.ascontiguousarray(np.asarray(inputs[n], dtype=np.float32))
        for n in names
    }

    nc = _get_program()
    in_maps = [dict(shared, x=x[c]) for c in range(N_CORES)]

    from concourse.bass_utils import run_bass_kernel_spmd
    res = run_bass_kernel_spmd(nc, in_maps, core_ids=list(range(N_CORES)))
    y = np.stack([res.results[c]["y"] for c in range(N_CORES)], axis=0)
    return y.astype(np.float32)


if __name__ == "__main__":
    rng = np.random.default_rng(0)
    demo = {
        "x": rng.standard_normal((B_FULL, S_FULL, D), dtype=np.float32),
        "Wq": rng.standard_normal((D, D), dtype=np.float32) * 0.02,
        "bq": np.zeros(D, np.float32),
        "Wk": rng.standard_normal((D, D), dtype=np.float32) * 0.02,
        "bk": np.zeros(D, np.float32),
        "Wv": rng.standard_normal((D, D), dtype=np.float32) * 0.02,
        "bv": np.zeros(D, np.float32),
        "gamma": np.ones(D, np.float32),
        "beta": np.zeros(D, np.float32),
    }
    out = kernel(**demo)
    print("kernel output", out.shape, out.dtype, float(np.abs(out).max()))

